# revision 40
# baseline (speedup 1.0000x reference)
"""Expert-parallel MoE FFN kernel for Trainium2 (8 NeuronCores).

Strategy: each of the 8 experts lives on its own core. Rows are routed
host-side (argsort by note_type_pos), padded to a uniform capacity C,
and shipped feature-major (transposed) in bf16 so the device kernel is
a pure dense 2-layer MLP with features on SBUF partitions:

    hT = relu(W1.T @ xT + b1)     [F, C]
    yT = W2.T @ hT + b2           [H, C]

In bf16 both expert weight matrices (16MB) fit in SBUF, so weights are
loaded once with a handful of coarse DMAs (the HWDGE charges a fixed
625ns per dma_start, so few/large transfers matter more than bytes) and
stay resident. The row dimension is split into <=512-wide tiles; for
each tile, layer 1 runs 32 fc-groups (8 k-matmuls PSUM-accumulated,
then a fused relu+bias to bf16 h), and layer 2 runs 8 m-groups that
accumulate all 32 fc contributions in a single PSUM bank before one
fused bias+identity drains to f32 and DMAs out. No cross-fb SBUF
accumulation exists, so the vector engine is idle and the drain after
the last matmul is just one activation + one small DMA.
"""

import sys

sys.path.insert(0, "/opt/trn_rl_repo")

import numpy as np

import concourse.bass as bass
import concourse.mybir as mybir
from concourse import bacc
from concourse.tile import TileContext

H = 1024
F = 4096
N_EXPERTS = 8
P = 128
KH = H // P    # 8   (H partition blocks = layer-2 output blocks m)
KF = F // P    # 32  (F partition blocks = layer-1 output groups fc)
G1 = 512       # W1 DMA group width (f columns per load)
NG1 = F // G1  # 8

NP_BF16 = mybir.dt.np(mybir.dt.bfloat16)


def _row_tiles(C):
    """Split C columns into chunks <=512 (multiples of 16). The first tile
    is narrowed to 288: its x bytes sit inside the critical startup chain
    (w1-head + x[tile0] + w1-q1 + sem gates the whole PE stream), and 288
    still keeps tile0's layer-1/layer-2 compute long enough to cover the
    W1/W2 weight-stream tails."""
    t0 = 288 if C >= 640 else 0
    rest = C - t0
    n = -(-rest // 512)
    rw = -(-rest // n)
    rw = ((rw + 15) // 16) * 16
    tiles = [(0, t0)] if t0 else []
    s = t0
    while s < C:
        w = min(rw, C - s)
        tiles.append((s, w))
        s += w
    return tiles


def build_expert_kernel(C, reps=1):
    """One expert's 2-layer MLP: xT [H, C] -> yT [H, C], weights resident."""
    f32 = mybir.dt.float32
    bf16 = mybir.dt.bfloat16
    nc = bacc.Bacc(None, target_bir_lowering=False)
    # Host-packed DRAM layouts, all chosen so every DMA's innermost
    # contiguous run is >=512B (full DMA-engine rate):
    #   xT  [128p, 8k, C]      xT[p,k,c]   = x[c, k*128+p]
    #   w1  [128p, 8k, F]      w1[p,k,f]   = W1[k*128+p, f]
    #   w2  [8m, 128p, 32fc, 128]  w2[m,p,fc,c] = W2[fc*128+p, m*128+c]
    xT = nc.dram_tensor("xT", [P, KH, C], bf16, kind="ExternalInput")
    w1 = nc.dram_tensor("w1", [P, KH, F], bf16, kind="ExternalInput")
    w2 = nc.dram_tensor("w2", [KH, P, KF, P], bf16, kind="ExternalInput")
    b1v = nc.dram_tensor("b1v", [P, KF], f32, kind="ExternalInput")
    b2v = nc.dram_tensor("b2v", [P, KH], f32, kind="ExternalInput")
    yT = nc.dram_tensor("yT", [H, C], bf16, kind="ExternalOutput")

    tiles = _row_tiles(C)
    wmax = max(t[1] for t in tiles)
    lw_f = 64 if tiles[-1][1] > 96 else tiles[-1][1] // 2

    with TileContext(nc) as tc:
        with (
            tc.tile_pool(name="consts", bufs=1) as consts,
            tc.tile_pool(name="xp", bufs=1) as xp,
            tc.tile_pool(name="w1p", bufs=1) as w1p,
            tc.tile_pool(name="w2p", bufs=1) as w2p,
            tc.tile_pool(name="hp", bufs=1) as hp,
            tc.tile_pool(name="yst", bufs=4) as yst,
            tc.tile_pool(name="psh", bufs=4, space="PSUM") as psh,
            tc.tile_pool(name="psy", bufs=4, space="PSUM") as psy,
        ):
            b1_sb = consts.tile([P, KF], f32, tag="b1")
            b2_sb = consts.tile([P, KH], f32, tag="b2")
            x_sb = xp.tile([P, KH, C], bf16, tag="xT")
            w1_sb = w1p.tile([P, KH, F], bf16, tag="w1")
            w2_sb = w2p.tile([P, KH, KF, P], bf16, tag="w2")

            # Startup-critical DMA order (HWDGE charges 625ns per issue and
            # DMA_ENGINES transfers strictly in order): W1's first fc block
            # and x[tile0] (split so the first k-matmuls start on the first
            # half) lead; W1 streams in groups sized to stay just ahead of
            # the fc-group consumption cadence; W2 follows with b2 ahead of
            # the first layer-2 activation.
            r0_, rw_ = tiles[0]

            def w1_load(c0, c1):
                nc.sync.dma_start(w1_sb[:, :, c0:c1], w1[:, :, c0:c1])

            w1_load(0, 2 * P)
            for ka, kb in ((0, 3), (3, 6), (6, 8)):
                nc.sync.dma_start(x_sb[:, ka:kb, r0_:r0_ + rw_],
                                  xT[:, ka:kb, r0_:r0_ + rw_])
            w1_load(2 * P, 4 * P)
            nc.sync.dma_start(b1_sb[:], b1v[:, :])
            for q in range(2, KF // 2):    # 256-col groups: cols 512..4096
                w1_load(q * 2 * P, (q + 1) * 2 * P)
            if len(tiles) > 1:
                nc.sync.dma_start(x_sb[:, :, rw_:C], xT[:, :, rw_:C])
            nc.sync.dma_start(b2_sb[:], b2v[:, :])
            for m in range(KH):
                nc.sync.dma_start(w2_sb[:, m, :, :], w2[m, :, :, :])

            # PE warmup: absorbs the p-state ramp during the startup DMA
            # window so real matmuls start at full clock; sized to end just
            # as the first fc-group's data lands.
            wu = consts.tile([P, 512], f32, tag="wu")
            nc.vector.memset(wu[:], 0.0)
            # dummy activation: pulls the 1.3us LoadActFuncSet into the idle
            # startup window so the first real relu isn't delayed by it
            actwu = consts.tile([P, 1], f32, tag="actwu")
            nc.scalar.activation(actwu[:], wu[:, 0:1],
                                 mybir.ActivationFunctionType.Relu)
            wups = psh.tile([P, 512], f32, tag="ph")
            for w_ in (416, 416, 304):
                nc.tensor.matmul(wups[:, 0:w_], wu[:, 0:P], wu[:, 0:w_],
                                 start=True, stop=True)

            h_sb = hp.tile([P, KF, wmax], bf16, tag="h")

            def l1_relu(fc, ph, rw):
                nc.scalar.activation(
                    h_sb[:, fc, :rw], ph[:],
                    mybir.ActivationFunctionType.Relu,
                    bias=b1_sb[:, fc:fc + 1])

            def l1_part(fc, ph, rw, r0, ks, ke, start, stop):
                for k in range(ks, ke):
                    nc.tensor.matmul(
                        ph[:],
                        w1_sb[:, k, fc * P:(fc + 1) * P],
                        x_sb[:, k, r0:r0 + rw],
                        start=(start and k == ks), stop=(stop and k == ke - 1))

            def layer1(r0, rw, cold=False):
                fc0 = 0
                if cold:
                    # Startup: x[tile0] lands in three k-chunks ~0.75us
                    # apart. Run the matching k-partials for the first two
                    # fc-groups against each chunk as it arrives (held-open
                    # PSUM groups) — absorbs the x DMA tail entirely.
                    pha = psh.tile([P, rw], f32, tag="ph")
                    phb = psh.tile([P, rw], f32, tag="ph")
                    for ka, kb in ((0, 3), (3, 6), (6, KH)):
                        l1_part(0, pha, rw, r0, ka, kb, ka == 0, kb == KH)
                        l1_part(1, phb, rw, r0, ka, kb, ka == 0, kb == KH)
                    l1_relu(0, pha, rw)
                    l1_relu(1, phb, rw)
                    fc0 = 2
                for fc in range(fc0, KF):
                    ph = psh.tile([P, rw], f32, tag="ph")
                    l1_part(fc, ph, rw, r0, 0, KH, True, True)
                    l1_relu(fc, ph, rw)

            def l2_group(m, r0, l0, lw, last):
                # r0: tile's global column offset (for yT); l0: local column
                # offset within the tile's h slab.
                py = psy.tile([P, lw], f32, tag="py")
                for fc in range(KF):
                    nc.tensor.matmul(
                        py[:],
                        w2_sb[:, m, fc, :],
                        h_sb[:, fc, l0:l0 + lw],
                        start=(fc == 0), stop=(fc == KF - 1))
                yo = yst.tile([P, wmax], bf16, tag="yo")
                nc.scalar.activation(
                    yo[:, :lw], py[:],
                    mybir.ActivationFunctionType.Identity,
                    bias=b2_sb[:, m:m + 1])
                if last:
                    nc.sync.dma_start(
                        yT[m * P:(m + 1) * P, r0 + l0:r0 + l0 + lw],
                        yo[:, :lw])

            def layer2(r0, rw, last, final):
                for m in range(KH):
                    if final and m == KH - 1:
                        # Split the kernel's very last m-group so the tail
                        # act->DMA->sem chain drains only a small chunk.
                        l2_group(m, r0, 0, rw - lw_f, last)
                        l2_group(m, r0, rw - lw_f, lw_f, last)
                    else:
                        l2_group(m, r0, 0, rw, last)

            for rep in range(reps):
                last = rep == reps - 1
                for ti, (r0, rw) in enumerate(tiles):
                    layer1(r0, rw, cold=(rep == 0 and ti == 0))
                    layer2(r0, rw, last, last and ti == len(tiles) - 1)
    nc.finalize()
    return nc


# SBUF residency: weights 128KB/partition + x (2*KH*C B) + h (2*KF*512 B).
MAX_C = 1536


def _prepare(x, note_type_pos, W1, b1, W2, b2, cap):
    """Host-side routing: sort rows by expert, pad to capacity C (<= cap),
    pack per-expert tensors into the kernel's DMA-friendly layouts."""
    ntp = np.asarray(note_type_pos).astype(np.int64)
    x = np.ascontiguousarray(np.asarray(x, dtype=np.float32))
    counts = np.bincount(ntp, minlength=N_EXPERTS)
    C = min(int(counts.max()), cap)
    C = max(16, ((C + 15) // 16) * 16)

    order = np.argsort(ntp, kind="stable")
    weights = []
    for e in range(N_EXPERTS):
        w1e = np.asarray(W1[e], dtype=np.float32).astype(NP_BF16)
        w2e = np.asarray(W2[e], dtype=np.float32).astype(NP_BF16)
        weights.append({
            # [128p, 8k, F]
            "w1": np.ascontiguousarray(
                w1e.reshape(KH, P, F).transpose(1, 0, 2)),
            # [8m, 128p, 32fc, 128c]
            "w2": np.ascontiguousarray(
                w2e.reshape(KF, P, KH, P).transpose(2, 1, 0, 3)),
            "b1v": np.ascontiguousarray(
                np.asarray(b1[e], dtype=np.float32).reshape(KF, P).T),
            "b2v": np.ascontiguousarray(
                np.asarray(b2[e], dtype=np.float32).reshape(KH, P).T),
        })
    launches = []
    off = 0
    expert_rows = []
    for e in range(N_EXPERTS):
        expert_rows.append(order[off:off + counts[e]])
        off += counts[e]
    n_launch = max(1, -(-int(counts.max()) // C))
    for g in range(n_launch):
        in_maps, row_idx = [], []
        for e in range(N_EXPERTS):
            rows = expert_rows[e][g * C:(g + 1) * C]
            row_idx.append(rows)
            xe = np.zeros((C, H), dtype=np.float32)
            if len(rows):
                xe[:len(rows)] = x[rows]
            # [128p, 8k, C]
            xpack = np.ascontiguousarray(
                xe.T.astype(NP_BF16).reshape(KH, P, C).transpose(1, 0, 2))
            in_maps.append({"xT": xpack, **weights[e]})
        launches.append((in_maps, row_idx))
    return launches, C


def kernel(x, note_type_pos, W1, b1, W2, b2):
    launches, C = _prepare(x, note_type_pos, W1, b1, W2, b2, cap=MAX_C)
    nc = build_expert_kernel(C)
    from concourse.bass_utils import run_bass_kernel_spmd
    T = np.asarray(x).shape[0]
    out = np.zeros((T, H), dtype=np.float32)
    for in_maps, row_idx in launches:
        res = run_bass_kernel_spmd(nc, in_maps, core_ids=list(range(N_EXPERTS)))
        for e in range(N_EXPERTS):
            rows = row_idx[e]
            if len(rows):
                out[rows] = res.results[e]["yT"].astype(np.float32).T[:len(rows)]
    return out
